# revision 33
# baseline (speedup 1.0000x reference)
"""Distributed Trainium2 Bass kernel for nn_Attention_13125420057022.

Multi-head attention (B=2, S=2048, H=768, 12 heads, head_dim=64) with
interleaved RoPE, run SPMD on 8 NeuronCores.

Sharding: core c handles batch b=c//4 and query rows [512*(c%4), 512*(c%4+1)).
Data-parallel with replicated K/V (redundant K/V compute beats the measured
~90us 4-rank AllGather). No collectives.

v2 restructure vs the first working kernel (258us):
- Scores matmuls for the two heads of a kT block are emitted back-to-back as
  64-row PE tiles at base partitions 0/64, so the array runs them
  concurrently (row tiling) instead of at half utilization.
- The attention pipeline is interleaved with the QKV projections: Q^T first
  (fills the input-DMA window), then per head-pair waves of
  K^T(t) -> scores(t) -> exp(t), with the V projection and A.V of the
  previous pair round-robined between score chunks. ACT (the exp engine,
  ~110us of work) starts ~15us into the kernel instead of ~110us, and A.V
  matmuls never sit in TensorE's FIFO waiting on exp.
- V bias is applied on DVE in the softmax epilogue (b_d folds out of the
  normalized context sum), dropping 32 rank-1 bias matmuls.
- Input diet: xTq dropped (Q reads the xT slice), rope tables shipped as
  [32/64, S] seeds and expanded on-chip; queue assignment spreads the ~8.3MB
  input load across sync/vector/gpsimd queues in dependency order, and the
  scalar queue carries only the early rope tables so ACT never does queue
  work once exps start.

Compute is bf16 with f32 PSUM accumulation. Scores are computed transposed
(S^T[k,q]) so exp output feeds A.V directly; softmax row-sums ride a ones
column appended to V (M=65); no max-subtraction needed (bounded logits).
Per-query 1/sum broadcast on GpSimd; RoPE uses a host-side de-interleave
permutation of Q/K weight rows (rotate-half via 32-row SB->SB DMA swaps).
"""

import math
import sys
from contextlib import ExitStack

import numpy as np
import ml_dtypes

sys.path.insert(0, "/opt/trn_rl_repo")

import concourse.bass as bass  # noqa: E402
import concourse.mybir as mybir  # noqa: E402
import concourse.tile as tile  # noqa: E402
from concourse import bacc  # noqa: E402
from concourse.bass_utils import run_bass_kernel_spmd  # noqa: E402

BF16 = ml_dtypes.bfloat16
F32 = mybir.dt.float32
BF = mybir.dt.bfloat16

B, S, H = 2, 2048, 768
NH, HD = 12, 64
THETA = 10000.0
NCORES = 8
GROUP = 4  # cores per batch
SLOC = S // GROUP  # 512 query rows per core
NKB = S // 128  # 16 key blocks

EXP = mybir.ActivationFunctionType.Exp


def build_graph():
    nc = bacc.Bacc(
        "TRN2",
        target_bir_lowering=False,
        debug=False,
        num_devices=NCORES,
    )

    xT = nc.dram_tensor("xT", [H, S], BF, kind="ExternalInput")
    xTq = nc.dram_tensor("xTq", [H, SLOC], BF, kind="ExternalInput")
    wt = nc.dram_tensor("wt", [H, 3 * H], BF, kind="ExternalInput")
    qkvb_qk = nc.dram_tensor("qkvb_qk", [128, 12], F32, kind="ExternalInput")
    biasv = nc.dram_tensor("biasv", [64, 12], F32, kind="ExternalInput")
    projt = nc.dram_tensor("projt", [H, H], BF, kind="ExternalInput")
    projb = nc.dram_tensor("projb", [1, H], BF, kind="ExternalInput")
    ckc = nc.dram_tensor("ckc", [32, S], BF, kind="ExternalInput")
    skc = nc.dram_tensor("skc", [64, S], BF, kind="ExternalInput")
    cqc = nc.dram_tensor("cqc", [32, SLOC], BF, kind="ExternalInput")
    sqc = nc.dram_tensor("sqc", [64, SLOC], BF, kind="ExternalInput")
    out_ext = nc.dram_tensor("out", [SLOC, H], F32, kind="ExternalOutput")

    with tile.TileContext(nc) as tc, ExitStack() as ctx:
        singles = ctx.enter_context(tc.tile_pool(name="singles", bufs=1))
        kT_p = ctx.enter_context(tc.tile_pool(name="kTp", bufs=3))
        kraw_p = ctx.enter_context(tc.tile_pool(name="kraw", bufs=2))
        kswp_p = ctx.enter_context(tc.tile_pool(name="kswp", bufs=2))
        ktmp_p = ctx.enter_context(tc.tile_pool(name="ktmp", bufs=1))
        qraw_p = ctx.enter_context(tc.tile_pool(name="qraw", bufs=2))
        qswp_p = ctx.enter_context(tc.tile_pool(name="qswp", bufs=2))
        qtmp_p = ctx.enter_context(tc.tile_pool(name="qtmp", bufs=1))
        v_pool = ctx.enter_context(tc.tile_pool(name="v_pool", bufs=1))
        at_pool = ctx.enter_context(tc.tile_pool(name="at", bufs=18))
        small_p = ctx.enter_context(tc.tile_pool(name="small", bufs=2))
        ctxn_p = ctx.enter_context(tc.tile_pool(name="ctxn", bufs=2))
        out_p = ctx.enter_context(tc.tile_pool(name="outp", bufs=2))

        # ---- SBUF singles ----
        wt_sb = singles.tile([128, 6, 3 * H], BF)
        xT_sb = singles.tile([128, 6, S], BF)
        projt_sb = singles.tile([128, 6, H], BF)
        projb_sb = singles.tile([1, H], BF)
        qkvb_sb = singles.tile([128, 12], F32)
        biasv_sb = singles.tile([64, 12], F32)
        ck_sb = singles.tile([128, S], BF)
        sk_sb = singles.tile([128, S], BF)
        cq_sb = singles.tile([128, SLOC], BF)
        sq_sb = singles.tile([128, SLOC], BF)
        ones_bf = singles.tile([1, 128], BF)
        qT_sb = singles.tile([128, 6, SLOC], BF)
        xTq_sb = singles.tile([128, 6, SLOC], BF)
        ctxT_sb = singles.tile([128, 6, SLOC], BF)

        wt_r = wt.ap().rearrange("(c p) n -> c p n", p=128)
        xT_r = xT.ap().rearrange("(c p) s -> c p s", p=128)
        xTq_r = xTq.ap().rearrange("(c p) s -> c p s", p=128)
        projt_r = projt.ap().rearrange("(c p) n -> c p n", p=128)

        # ---- input DMA, in dependency order ----
        # sync queue: own xT chunk (Q needs it first), then xT c=0..2
        for c in range(6):
            nc.sync.dma_start(out=xTq_sb[:, c, :], in_=xTq_r[c])
        for c in range(3):
            for sc in range(4):
                nc.sync.dma_start(
                    out=xT_sb[:, c, sc * 512 : (sc + 1) * 512],
                    in_=xT_r[c][:, sc * 512 : (sc + 1) * 512],
                )
        # scalar queue: rope seeds + expansion first, then xT c=3..5, then proj
        nc.scalar.dma_start(out=ck_sb[0:32, :], in_=ckc.ap())
        nc.scalar.dma_start(out=sk_sb[0:64, :], in_=skc.ap())
        nc.scalar.dma_start(out=cq_sb[0:32, :], in_=cqc.ap())
        nc.scalar.dma_start(out=sq_sb[0:64, :], in_=sqc.ap())
        nc.scalar.dma_start(out=cq_sb[32:64, :], in_=cq_sb[0:32, :])
        nc.scalar.dma_start(out=cq_sb[64:128, :], in_=cq_sb[0:64, :])
        nc.scalar.dma_start(out=sq_sb[64:128, :], in_=sq_sb[0:64, :])
        nc.scalar.dma_start(out=ck_sb[32:64, :], in_=ck_sb[0:32, :])
        nc.scalar.dma_start(out=ck_sb[64:128, :], in_=ck_sb[0:64, :])
        nc.scalar.dma_start(out=sk_sb[64:128, :], in_=sk_sb[0:64, :])
        for c in range(3, 6):
            for sc in range(4):
                nc.scalar.dma_start(
                    out=xT_sb[:, c, sc * 512 : (sc + 1) * 512],
                    in_=xT_r[c][:, sc * 512 : (sc + 1) * 512],
                )
        for c in range(6):
            nc.scalar.dma_start(out=projt_sb[:, c, :], in_=projt_r[c])
        nc.scalar.dma_start(out=projb_sb[:], in_=projb.ap())
        # gpsimd queue: biases + Q weights now (t0's blocks first so Q0 can
        # start immediately); K/V weights emitted after the Q waves so the Q
        # rope-swap DMAs land ahead of them in the queue.
        nc.gpsimd.dma_start(out=qkvb_sb[:], in_=qkvb_qk.ap())
        nc.gpsimd.dma_start(out=biasv_sb[:], in_=biasv.ap())
        for c in range(6):
            nc.gpsimd.dma_start(out=wt_sb[:, c, 0:128], in_=wt_r[c][:, 0:128])
        for c in range(6):
            nc.gpsimd.dma_start(out=wt_sb[:, c, 128:768], in_=wt_r[c][:, 128:768])
        nc.vector.memset(ones_bf[:], 1.0)

        def rope(dst, raw, swp, cos_sb, sin_sb, tmp_pool, width):
            """dst = raw*cos + swp*sin (rotate-half form, swp pre-swapped)."""
            t1 = tmp_pool.tile([128, width], BF, tag="t1")
            t2 = tmp_pool.tile([128, width], BF, tag="t2")
            nc.vector.tensor_mul(t1[:], raw[:], cos_sb)
            nc.vector.tensor_mul(t2[:], swp[:], sin_sb)
            nc.vector.tensor_add(dst, t1[:], t2[:])

        def swap_dmas(q, swp, raw, width):
            q.dma_start(out=swp[0:32, 0:width], in_=raw[32:64, 0:width])
            q.dma_start(out=swp[32:64, 0:width], in_=raw[0:32, 0:width])
            q.dma_start(out=swp[64:96, 0:width], in_=raw[96:128, 0:width])
            q.dma_start(out=swp[96:128, 0:width], in_=raw[64:96, 0:width])

        with (
            tc.tile_pool(name="s_ps", bufs=2, space="PSUM") as s_ps,
            tc.tile_pool(name="kqv_ps", bufs=2, space="PSUM") as kqv_ps,
            tc.tile_pool(name="ctx_ps", bufs=2, space="PSUM") as ctx_psum,
        ):
            # ---- Q^T for own rows, all six blocks up front ----
            for t in range(6):
                qps = kqv_ps.tile([128, 512], F32, tag="kv")
                for c in range(6):
                    nc.tensor.matmul(
                        qps[:],
                        lhsT=wt_sb[:, c, t * 128 : (t + 1) * 128],
                        rhs=xTq_sb[:, c, :],
                        start=(c == 0),
                        stop=(c == 5),
                    )
                qraw = qraw_p.tile([128, SLOC], BF)
                nc.vector.tensor_scalar_add(qraw[:], qps[:], qkvb_sb[:, t : t + 1])
                qswp = qswp_p.tile([128, SLOC], BF)
                swap_dmas(nc.gpsimd, qswp, qraw, SLOC)
                rope(qT_sb[:, t, :], qraw, qswp, cq_sb[:], sq_sb[:], qtmp_p, SLOC)

            # K weights (t0 first), then V weights — queued behind the Q swaps
            for c in range(6):
                nc.gpsimd.dma_start(out=wt_sb[:, c, 768:896], in_=wt_r[c][:, 768:896])
            for c in range(6):
                nc.gpsimd.dma_start(out=wt_sb[:, c, 1536:2304], in_=wt_r[c][:, 1536:2304])
            for c in range(6):
                nc.gpsimd.dma_start(out=wt_sb[:, c, 896:1536], in_=wt_r[c][:, 896:1536])

            kts = {}
            v_tiles = [None] * NKB
            ctx_tiles = {}

            def k_block(t):
                nb = t + 6
                kt = kT_p.tile([128, S], BF, tag="kt", name=f"kt{t}")
                kts[t] = kt
                for sp in range(2):
                    kraw = kraw_p.tile([128, 1024], BF)
                    for half in range(2):
                        sc = 2 * sp + half
                        kps = kqv_ps.tile([128, 512], F32, tag="kv", name="kps")
                        for c in range(6):
                            nc.tensor.matmul(
                                kps[:],
                                lhsT=wt_sb[:, c, nb * 128 : (nb + 1) * 128],
                                rhs=xT_sb[:, c, sc * 512 : (sc + 1) * 512],
                                start=(c == 0),
                                stop=(c == 5),
                            )
                        nc.vector.tensor_scalar_add(
                            kraw[:, half * 512 : (half + 1) * 512],
                            kps[:],
                            qkvb_sb[:, nb : nb + 1],
                        )
                    kswp = kswp_p.tile([128, 1024], BF)
                    swap_dmas(nc.sync, kswp, kraw, 1024)
                    cs = slice(sp * 1024, (sp + 1) * 1024)
                    rope(kt[:, cs], kraw, kswp, ck_sb[:, cs], sk_sb[:, cs], ktmp_p, 1024)

            def s_chunk(t, i):
                """Scores + exp for head pair t, key blocks 2i, 2i+1 (both heads).

                The two heads' matmuls are emitted alternately so the 64-row
                PE tiles at base partitions 0 and 64 run concurrently."""
                kt = kts[t]
                stA = s_ps.tile([128, 1024], F32, tag="st", name="stA")
                stB = s_ps.tile([128, 1024], F32, tag="st", name="stB")
                for half in range(2):
                    kb = 2 * i + half
                    ks = slice(kb * 128, (kb + 1) * 128)
                    os_ = slice(half * 512, (half + 1) * 512)
                    nc.tensor.matmul(
                        stA[:, os_], lhsT=kt[0:64, ks], rhs=qT_sb[0:64, t, :],
                        start=True, stop=True,
                    )
                    nc.tensor.matmul(
                        stB[:, os_], lhsT=kt[64:128, ks], rhs=qT_sb[64:128, t, :],
                        start=True, stop=True,
                    )
                atA = at_pool.tile([128, 1024], BF, tag="at", name="atA")
                atB = at_pool.tile([128, 1024], BF, tag="at", name="atB")
                nc.scalar.activation(out=atA[:], in_=stA[:], func=EXP)
                nc.scalar.activation(out=atB[:], in_=stB[:], func=EXP)
                return atA, atB

            def v_block(i):
                vps_a = kqv_ps.tile([128, 512], F32, tag="kv", name="vps_a")
                vps_b = kqv_ps.tile([128, 512], F32, tag="kv", name="vps_b")
                for c in range(6):
                    lhsT = xT_sb[:, c, i * 128 : (i + 1) * 128]
                    nc.tensor.matmul(
                        vps_a[:], lhsT=lhsT, rhs=wt_sb[:, c, 1536:2048],
                        start=(c == 0), stop=(c == 5),
                    )
                    nc.tensor.matmul(
                        vps_b[:, 0:256], lhsT=lhsT, rhs=wt_sb[:, c, 2048:2304],
                        start=(c == 0), stop=(c == 5),
                    )
                vt = v_pool.tile([128, NH * 65], BF, tag=f"v{i}", name=f"vt{i}")
                vt3 = vt.rearrange("p (h c) -> p h c", h=NH)
                nc.vector.tensor_copy(
                    vt3[:, 0:8, 0:64], vps_a.rearrange("p (h d) -> p h d", h=8)
                )
                nc.vector.tensor_copy(
                    vt3[:, 8:12, 0:64],
                    vps_b[:, 0:256].rearrange("p (h d) -> p h d", h=4),
                )
                nc.vector.memset(vt3[:, :, 64:65], 1.0)
                v_tiles[i] = vt

            def av_chunk(p, i, ats):
                """A.V for head pair p over key blocks 2i, 2i+1."""
                if i == 0:
                    ctx_tiles[p] = (
                        ctx_psum.tile([65, SLOC], F32, tag="ctx", name="ctxA"),
                        ctx_psum.tile([65, SLOC], F32, tag="ctx", name="ctxB"),
                    )
                ctxA, ctxB = ctx_tiles[p]
                atA, atB = ats
                hA, hB = 2 * p, 2 * p + 1
                for half in range(2):
                    kb = 2 * i + half
                    os_ = slice(half * 512, (half + 1) * 512)
                    nc.tensor.matmul(
                        ctxA[:], lhsT=v_tiles[kb][:, hA * 65 : (hA + 1) * 65],
                        rhs=atA[:, os_], start=(kb == 0), stop=(kb == NKB - 1),
                    )
                    nc.tensor.matmul(
                        ctxB[:], lhsT=v_tiles[kb][:, hB * 65 : (hB + 1) * 65],
                        rhs=atB[:, os_], start=(kb == 0), stop=(kb == NKB - 1),
                    )

            def epilogue(p):
                ctxA, ctxB = ctx_tiles[p]
                for r0, ctxp, h in ((0, ctxA, 2 * p), (64, ctxB, 2 * p + 1)):
                    sums64 = small_p.tile([128, SLOC], F32, tag="sums64")
                    nc.vector.tensor_copy(sums64[64:65, :], ctxp[64:65, :])
                    sums = small_p.tile([1, SLOC], F32, tag="sums")
                    nc.sync.dma_start(out=sums[:], in_=sums64[64:65, :])
                    rec = small_p.tile([1, SLOC], F32, tag="rec")
                    nc.vector.reciprocal_approx_fast(out=rec[:], in_=sums[:])
                    bc_sb = small_p.tile([64, SLOC], F32, tag="bc_sb")
                    nc.gpsimd.partition_broadcast(bc_sb[:], rec[:], channels=64)
                    cn = ctxn_p.tile([64, SLOC], BF, tag="cn")
                    nc.vector.tensor_mul(cn[:], ctxp[0:64, :], bc_sb[:])
                    if r0 == 0:
                        nc.vector.tensor_scalar_add(
                            ctxT_sb[0:64, p, :], cn[:], biasv_sb[:, h : h + 1]
                        )
                    else:
                        cnb = ctxn_p.tile([64, SLOC], BF, tag="cnb")
                        nc.vector.tensor_scalar_add(
                            cnb[:], cn[:], biasv_sb[:, h : h + 1]
                        )
                        nc.sync.dma_start(out=ctxT_sb[64:128, p, :], in_=cnb[:])

            # ---- waves: K(t), then score chunks round-robined with V blocks
            # and a paced A.V cursor one pair behind (keeps TensorE fed while
            # ACT works through the exps, without at-pool deadlock) ----
            at_saved = {}
            state = {"av": 0, "v": 0}

            def pump_v(n):
                while state["v"] < min(n, NKB):
                    v_block(state["v"])
                    state["v"] += 1

            def pump_av(upto):
                while state["av"] < upto:
                    p, i = state["av"] // 8, state["av"] % 8
                    pump_v(2 * i + 2)  # A.V needs v_tiles[2i, 2i+1] emitted first
                    av_chunk(p, i, at_saved[(p, i)])
                    del at_saved[(p, i)]
                    state["av"] += 1
                    if i == 7:
                        epilogue(p)

            v_sched = [8, 12, 16, 16, 16, 16]  # cumulative V blocks by wave end
            for t in range(6):
                k_block(t)
                v_lo = v_sched[t - 1] if t else 0
                for i in range(8):
                    pump_v(v_lo + (v_sched[t] - v_lo) * (i + 1) // 8)
                    if t >= 1:
                        pump_av(min(8 * t, 8 * (t - 1) + i + 1))
                    at_saved[(t, i)] = s_chunk(t, i)
            pump_av(48)

        # ---- output projection: out[s,:] = ctx^T.T @ projt + projb ----
        with tc.tile_pool(name="o_ps", bufs=2, space="PSUM") as o_psum:
            for i in range(4):
                ops = o_psum.tile([128, H], F32)
                for c in range(6):
                    lhsT = ctxT_sb[:, c, i * 128 : (i + 1) * 128]
                    nc.tensor.matmul(
                        ops[:, 0:512], lhsT=lhsT, rhs=projt_sb[:, c, 0:512],
                        start=(c == 0), stop=False,
                    )
                    nc.tensor.matmul(
                        ops[:, 512:768], lhsT=lhsT, rhs=projt_sb[:, c, 512:768],
                        start=(c == 0), stop=False,
                    )
                nc.tensor.matmul(
                    ops[:, 0:512], lhsT=ones_bf[:, 0:128], rhs=projb_sb[:, 0:512],
                    start=False, stop=True,
                )
                nc.tensor.matmul(
                    ops[:, 512:768], lhsT=ones_bf[:, 0:128], rhs=projb_sb[:, 512:768],
                    start=False, stop=True,
                )
                osb = out_p.tile([128, H], F32)
                nc.vector.tensor_copy(osb[:], ops[:])
                nc.sync.dma_start(
                    out=out_ext.ap()[i * 128 : (i + 1) * 128, :], in_=osb[:]
                )

    nc.compile()
    return nc


_PERM = np.concatenate([np.arange(0, HD, 2), np.arange(1, HD, 2)])


def prep_inputs(x, qkv_w, qkv_b, proj_w, proj_b):
    """Shard + lay out the full inputs into per-core input maps."""
    x = np.asarray(x, np.float32)
    qkv_w = np.asarray(qkv_w, np.float32)
    qkv_b = np.asarray(qkv_b, np.float32)
    proj_w = np.asarray(proj_w, np.float32)
    proj_b = np.asarray(proj_b, np.float32)

    # de-interleave permutation of q/k head dims (rows of qkv_w)
    Wp = qkv_w.copy()
    bp = qkv_b.copy()
    for sec in range(2):
        for h in range(NH):
            base = sec * H + h * HD
            Wp[base : base + HD] = qkv_w[base + _PERM]
            bp[base : base + HD] = qkv_b[base + _PERM]
    wt = np.ascontiguousarray(Wp.T).astype(BF16)  # [768, 2304]
    qkvb_qk = np.ascontiguousarray(bp[: 2 * H].reshape(12, 128).T).astype(np.float32)
    biasv = np.ascontiguousarray(qkv_b[2 * H :].reshape(12, 64).T).astype(np.float32)
    projt = np.ascontiguousarray(proj_w.T).astype(BF16)
    projb = proj_b.reshape(1, H).astype(BF16)

    inv_freq = 1.0 / (THETA ** (np.arange(0, HD, 2, dtype=np.float32) / HD))
    angles = np.arange(S, dtype=np.float32)[None, :] * inv_freq[:, None]  # [32, S]
    cos_t, sin_t = np.cos(angles), np.sin(angles)
    qscale = 1.0 / math.sqrt(HD)

    ckc = np.ascontiguousarray(cos_t).astype(BF16)  # [32, S]
    skc = np.concatenate([-sin_t, sin_t], axis=0).astype(BF16)  # [64, S]
    xT_full = [np.ascontiguousarray(x[b].T).astype(BF16) for b in range(B)]

    in_maps = []
    for c in range(NCORES):
        b, j = c // GROUP, c % GROUP
        sl = slice(j * SLOC, (j + 1) * SLOC)
        cos_j, sin_j = cos_t[:, sl] * qscale, sin_t[:, sl] * qscale
        cqc = np.ascontiguousarray(cos_j).astype(BF16)  # [32, SLOC]
        sqc = np.concatenate([-sin_j, sin_j], axis=0).astype(BF16)  # [64, SLOC]
        in_maps.append(
            {
                "xT": xT_full[b],
                "xTq": np.ascontiguousarray(xT_full[b][:, sl]),
                "wt": wt,
                "qkvb_qk": qkvb_qk,
                "biasv": biasv,
                "projt": projt,
                "projb": projb,
                "cqc": cqc,
                "sqc": sqc,
                "ckc": ckc,
                "skc": skc,
            }
        )
    return in_maps


_NC_CACHE = {}


def get_graph():
    if "nc" not in _NC_CACHE:
        _NC_CACHE["nc"] = build_graph()
    return _NC_CACHE["nc"]


def run(inputs, trace=False, **kw):
    nc = get_graph()
    in_maps = prep_inputs(**inputs)
    res = run_bass_kernel_spmd(nc, in_maps, core_ids=list(range(NCORES)), trace=trace, **kw)
    out = np.empty((B, S, H), np.float32)
    for c in range(NCORES):
        b, j = c // GROUP, c % GROUP
        out[b, j * SLOC : (j + 1) * SLOC, :] = res.results[c]["out"]
    return out, res


def kernel(**inputs):
    out, _ = run(inputs, trace=False)
    return out


if __name__ == "__main__":
    print("building graph...")
    nc = get_graph()
    print("graph built and compiled")
